# revision 58
# baseline (speedup 1.0000x reference)
"""Bark-style causal self-attention on 8 Trainium2 NeuronCores — v3.

Problem (hardcoded): B=8, S=1024, D=1024, H=16 heads, Hd=64, fp32 I/O.
    qkv = X @ W_attn + b_attn ; causal softmax(QK^T/8) @ V ; out @ W_out + b_out

Sharding: pure data parallelism — batch b -> core b. No collectives.

v3 (vs v2 baseline):
  - All matmul inputs in bf16 (inputs converted on host): halves HBM
    traffic and SBUF footprint; PSUM accumulation stays fp32.
  - Batched DMA: Q|K weights arrive as 8 contiguous [128,2048] row
    tiles resident all pass; output stores are single [128,1024]
    transfers; X^T materializes via 8 HWDGE transposing DMAs (no PE
    transposes / DVE evicts / PSUM bounce at startup). Rowsum-bounce
    stores ride the ACT queue, keeping SP for the bulk loads.
  - Attention: per (head, sk-tile) item = bank-aligned scores matmul
    halves into one 2-bank PSUM tile + mask matmul + ONE exp reading
    across both banks, then the PV half-matmuls; depth-2 lookahead.
    PSUM pools: pss 2x2-bank / pso 2x1 / psg 2x1 = 8 banks.
  - Causal diag mask on the PE: a second matmul accumulates a constant
    -1e9 strict-lower-triangular tile into the diag scores chunk, so
    exp() zeroes it — no extra cross-engine hop on the et chain.
  - Softmax denominators: V_aug carries a ones column per head, so PSUM
    row 64 of the PV accumulation is the rowsum; reciprocal'd rows are
    broadcast via a DRAM bounce (pairs 0-6) or an on-chip K=1 PE
    broadcast (last pair, bf16 rs) to keep the tail short.
  - qkT fills interleave into the attention loop as PE filler
    (per-pair pop budget); pair t fills pair t+2's tiles.
  - phases/"ablation" modes (proj/noout/noexp/nopv) build partial
    kernels for hardware phase attribution; only "all" is correct.
"""

import os
import sys

sys.path.insert(0, "/opt/trn_rl_repo")
os.environ.setdefault("MYCRO_LOCAL_CACHE", "1")

import numpy as np

B, S, D = 8, 1024, 1024
H, HD = 16, 64
P = 128
N_CORES = 8
ST = S // P  # 8 s-tiles
DT = D // P  # 8 d-tiles
HB = 512  # psum half (one bank of fp32)

_NC_CACHE = {}


def _build_nc(mm_dtype_name="bfloat16", reps=1, phases="all"):
    import contextlib

    import concourse.bacc as bacc
    import concourse.bass as bass
    import concourse.mybir as mybir
    import concourse.tile as tile
    from concourse.masks import make_identity, make_lower_triangular

    EXP = mybir.ActivationFunctionType.Exp

    f32 = mybir.dt.float32
    bf16 = mybir.dt.bfloat16
    mdt = getattr(mybir.dt, mm_dtype_name)

    nc = bacc.Bacc("TRN2", target_bir_lowering=False, debug=False)

    x_d = nc.dram_tensor("hidden_states", [S, D], mdt, kind="ExternalInput")
    wa_d = nc.dram_tensor("W_attn", [D, 3 * D], mdt, kind="ExternalInput")
    ba_d = nc.dram_tensor("b_attn", [3 * D], f32, kind="ExternalInput")
    wo_d = nc.dram_tensor("W_out", [D, D], mdt, kind="ExternalInput")
    bo_d = nc.dram_tensor("b_out", [D], f32, kind="ExternalInput")
    out_d = nc.dram_tensor("out", [S, D], f32, kind="ExternalOutput")
    # softmax denominator bounce rows (DRAM allows partition-broadcast reads)
    rows_dram = nc.dram_tensor("rows_bounce", [H, S], f32, kind="Internal")

    with tile.TileContext(nc) as tc:
        with contextlib.ExitStack() as pools:
            const = pools.enter_context(tc.tile_pool(name="const", bufs=1))
            r8 = pools.enter_context(tc.tile_pool(name="r8", bufs=16))
            qkp = pools.enter_context(tc.tile_pool(name="qkp", bufs=8))
            attp = pools.enter_context(tc.tile_pool(name="attp", bufs=8))
            vp = pools.enter_context(tc.tile_pool(name="vp", bufs=8))
            etp = pools.enter_context(tc.tile_pool(name="etp", bufs=6))
            rsp = pools.enter_context(tc.tile_pool(name="rsp", bufs=2))
            wqp = pools.enter_context(tc.tile_pool(name="wqp", bufs=8))
            obp = pools.enter_context(tc.tile_pool(name="obp", bufs=2))
            pso = pools.enter_context(tc.tile_pool(name="pso", bufs=2, space="PSUM"))
            pss = pools.enter_context(tc.tile_pool(name="pss", bufs=2, space="PSUM"))
            bcp = pools.enter_context(tc.tile_pool(name="bcp", bufs=1))
            psg = pools.enter_context(tc.tile_pool(name="psg", bufs=2, space="PSUM"))

            # ---- constants -------------------------------------------------
            identity_f = const.tile([P, P], f32, name="identity_f")
            make_identity(nc, identity_f)

            # per-channel bias for q/k channels as per-partition cols [128,16]
            bqk = const.tile([P, H], f32, name="bqk")
            nc.scalar.dma_start(
                out=bqk, in_=ba_d.ap().rearrange("(t p) -> p t", p=P)[:, 0:H]
            )
            # partition-broadcast bias rows for V and the output projection
            # (DMAs issued inside the startup loop, on the ACT queue)
            bias_v = const.tile([P, D], f32, name="bias_v")
            bias_o = const.tile([P, D], f32, name="bias_o")
            ones_f = const.tile([P, 64], f32, name="ones_f")
            nc.gpsimd.memset(ones_f, 1.0)
            onesv = const.tile([P, H], bf16, name="onesv")
            nc.vector.tensor_copy(onesv, ones_f[:, 0:H])
            ones_b = const.tile([P, 64], bf16, name="ones_b")
            nc.vector.tensor_copy(ones_b, ones_f)
            # causal diag-block mask, applied on the PE: a second matmul
            # accumulates trilneg (0 where q >= k, -1e9 strictly below the
            # diagonal) into the diag scores chunk through identity weights,
            # so exp(scale*(s - 1e9)) == 0 with no cross-engine hop.
            trilneg = const.tile([P, P], bf16, name="trilneg")
            nc.gpsimd.memset(trilneg, 0.0)
            nc.gpsimd.affine_select(
                out=trilneg,
                in_=trilneg,
                compare_op=mybir.AluOpType.is_ge,
                fill=-1e9,
                base=0,
                pattern=[[1, P]],
                channel_multiplier=-1,
            )
            identity_b = const.tile([P, P], bf16, name="identity_b")
            nc.vector.tensor_copy(identity_b, identity_f)

            def one_pass():
              # ---- startup: X load + transpose, V chases one s-tile behind --
              xt = []
              for d in range(DT):
                  xt.append(r8.tile([P, S], mdt, name=f"xt{d}", tag="r8"))
              wv = []

              v_aug = [None] * ST

              def emit_v(s):
                  # one 2-bank PSUM tile; k-outer c-inner halves share the
                  # stationary operand (one LDWEIGHTS per k)
                  ps_v = pss.tile([P, D], f32, name="ps_v", tag="pss")
                  for k in range(DT):
                      for c in range(2):
                          nc.tensor.matmul(
                              ps_v[:, c * HB : (c + 1) * HB],
                              xt[k][:, s * P : (s + 1) * P],
                              wv[k][:, c * HB : (c + 1) * HB],
                              start=(k == 0),
                              stop=(k == DT - 1),
                          )
                  va = vp.tile([P, H * 65], bf16, name=f"vaug{s}", tag="v")
                  va3 = va.rearrange("p (h c) -> p h c", c=65)
                  for c in range(2):
                      nc.vector.tensor_add(
                          va3[:, c * 8 : (c + 1) * 8, 0:64],
                          ps_v[:, c * HB : (c + 1) * HB].rearrange(
                              "p (h c) -> p h c", c=64
                          ),
                          bias_v[:, c * HB : (c + 1) * HB].rearrange(
                              "p (h c) -> p h c", c=64
                          ),
                      )
                  nc.vector.tensor_copy(va3[:, :, 64:65], onesv[:, :, None])
                  v_aug[s] = va

              # X arrives pre-transposed via HWDGE transposing DMAs (no PE
              # transposes / DVE evicts); W_v interleaved on the Pool queue.
              for d in range(DT):
                  (nc.sync if d % 2 == 0 else nc.scalar).dma_start(
                      out=xt[d],
                      in_=x_d[0:S, d * P : (d + 1) * P],
                      transpose=True,
                  )
                  w = r8.tile([P, D], mdt, name=f"wv{d}", tag="r8")
                  nc.gpsimd.dma_start(
                      out=w[:, 0:HB],
                      in_=wa_d[d * P : (d + 1) * P, 2 * D : 2 * D + HB],
                  )
                  wv.append(w)
              for k in range(DT):
                  (nc.sync if k % 2 == 0 else nc.scalar).dma_start(
                      out=wv[k][:, HB:D],
                      in_=wa_d[k * P : (k + 1) * P, 2 * D + HB : 3 * D],
                  )
              nc.gpsimd.dma_start(
                  out=bias_v,
                  in_=bass.AP(tensor=ba_d, offset=2 * D, ap=[[0, P], [1, D]]),
              )
              nc.gpsimd.dma_start(
                  out=bias_o,
                  in_=bass.AP(tensor=bo_d, offset=0, ap=[[0, P], [1, D]]),
              )
              for s in range(ST):
                  emit_v(s)

              # Q|K weights: 8 contiguous row-tiles [128, 2048], resident all
              # pass (bf16 leaves SBUF slack); split over the SP/ACT queues.
              wq = []
              for k in range(DT):
                  w = wqp.tile([P, 2 * D], mdt, name=f"wq{k}", tag="wq")
                  (nc.sync if k % 2 == 0 else nc.scalar).dma_start(
                      out=w, in_=wa_d[k * P : (k + 1) * P, 0 : 2 * D]
                  )
                  wq.append(w)

              # ---- qkT fill machinery --------------------------------------
              qkt = [None] * 2 * ST

              def make_pair_fills(t):
                  """Return PE/DVE thunks (one per matmul / evict) for pair
                  t's qkT tiles, to interleave later."""
                  thunks = []
                  for m in (t, 8 + t):
                      col0 = (m % 8) * P + (0 if m < 8 else D)
                      wsl = [wq[k][:, col0 : col0 + P] for k in range(DT)]
                      qkt[m] = qkp.tile([P, S], bf16, name=f"qkt{m}", tag="qk")
                      for c in range(2):
                          state = {}

                          for k in range(DT):
                              def mm_k(m=m, c=c, k=k, wsl=wsl, state=state):
                                  if k == 0:
                                      state["ps"] = psg.tile(
                                          [P, HB], f32, name="ps_g", tag="psg"
                                      )
                                  nc.tensor.matmul(
                                      state["ps"],
                                      wsl[k],
                                      xt[k][:, c * HB : (c + 1) * HB],
                                      start=(k == 0),
                                      stop=(k == DT - 1),
                                  )
                              thunks.append(mm_k)

                          def evict(m=m, c=c, state=state):
                              nc.vector.tensor_scalar_add(
                                  qkt[m][:, c * HB : (c + 1) * HB],
                                  state["ps"],
                                  bqk[:, m : m + 1],
                              )
                          thunks.append(evict)
                  return thunks

              class FillStream:
                  def __init__(self):
                      self.ops = []  # (consumer_pair, thunk)
                      self.i = 0
                      self.budget = 10**9

                  def add(self, ops, pair=-1):
                      self.ops.extend((pair, f) for f in ops)

                  def pop(self, n=1):
                      n = min(n, self.budget)
                      stop = min(self.i + n, len(self.ops))
                      while self.i < stop:
                          self.ops[self.i][1]()
                          self.i += 1
                          self.budget -= 1

                  def drain_through(self, pair):
                      """emit every fill whose consumer pair is <= pair"""
                      while self.i < len(self.ops) and self.ops[self.i][0] <= pair:
                          self.ops[self.i][1]()
                          self.i += 1

                  def drain(self):
                      self.pop(len(self.ops))

              fills = FillStream()
              fills.add(make_pair_fills(0), 0)
              fills.add(make_pair_fills(1), 1)
              fills.drain()

              # ---- attention -----------------------------------------------
              att = [None] * ST

              def emit_scores_j(t, po, j):
                  """scores for one (head, sk-tile j): 1 wide matmul (up to
                  1024-col bf16 moving operand, 2-bank PSUM) + 1 mask matmul
                  + 1 exp. et columns are relative to sq0."""
                  sq0 = j * P
                  w = S - sq0
                  et = etp.tile([P, w], bf16, name="et", tag="et")
                  ps_s = pss.tile([P, w], f32, name="ps_s", tag="pss")
                  # matmul halves split at the PSUM bank boundary (relative
                  # col 512); the single exp reads across both banks
                  for a in range(0, w, HB):
                      b = min(a + HB, w)
                      # bank-0 half stays open: the mask matmul closes it
                      nc.tensor.matmul(
                          ps_s[:, a:b],
                          qkt[8 + t][po : po + 64, sq0 : sq0 + P],
                          qkt[t][po : po + 64, sq0 + a : sq0 + b],
                          start=True,
                          stop=(a > 0),
                      )
                  nc.tensor.matmul(
                      ps_s[:, 0:P],
                      identity_b,
                      trilneg,
                      start=False,
                      stop=True,
                  )
                  nc.scalar.activation(et, ps_s, EXP, scale=0.125)
                  return et

              def emit_pv(t, h, j, et, pso_c):
                  sq0 = j * P
                  for c in range(2):
                      a = max(c * HB, sq0)
                      b = (c + 1) * HB
                      if a >= b:
                          continue
                      nc.tensor.matmul(
                          pso_c[c][0:65, a - c * HB : b - c * HB],
                          v_aug[j][:, h * 65 : h * 65 + 65],
                          et[:, a - sq0 : b - sq0],
                          start=(j == 0),
                          stop=(j == (3 if c == 0 else ST - 1)),
                      )

              def emit_evict(t, po, c, pso_c, rs):
                  """evict the c-half of one head's raw att^T + recip'd rowsum.
                  Head A's rows live at rs partition 0, head B's at 32 (both
                  legal quadrant starts)."""
                  cs = c * HB
                  rb = po // 2  # 0 or 32
                  with nc.allow_low_precision(reason="softmax denom recip"):
                      nc.vector.reciprocal(
                          rs[rb : rb + 1, cs : cs + HB], pso_c[c][64:65, 0:HB]
                      )
                  nc.vector.tensor_copy(
                      att[t][po : po + 64, cs : cs + HB], pso_c[c][0:64, 0:HB]
                  )

              def emit_pair_norm(t, rs):
                  """head rows -> DRAM, pair broadcast read, Pool multiply."""
                  for hh in range(2):
                      h = 2 * t + hh
                      nc.scalar.dma_start(
                          out=rows_dram[h : h + 1, :],
                          in_=rs[32 * hh : 32 * hh + 1, :],
                      )
                  bc = bcp.tile([P, S], f32, name="bc", tag="bc")
                  nc.gpsimd.dma_start(
                      out=bc,
                      in_=bass.AP(
                          tensor=rows_dram,
                          offset=2 * t * S,
                          ap=[[S, 2], [0, 64], [1, S]],
                      ),
                  )
                  nc.gpsimd.tensor_mul(att[t], att[t], bc)

              def emit_head_norm(t, hh, rs, bc):
                  """single-head normalize (used for the last pair so the
                  first head's multiply overlaps the second head's j-loop)"""
                  h = 2 * t + hh
                  po = 64 * hh
                  nc.scalar.dma_start(
                      out=rows_dram[h : h + 1, :],
                      in_=rs[32 * hh : 32 * hh + 1, :],
                  )
                  nc.gpsimd.dma_start(
                      out=bc[po : po + 64, :],
                      in_=bass.AP(
                          tensor=rows_dram, offset=h * S, ap=[[0, 64], [1, S]]
                      ),
                  )
                  nc.gpsimd.tensor_mul(
                      att[t][po : po + 64, :], att[t][po : po + 64, :],
                      bc[po : po + 64, :],
                  )

              wout = []
              last_norms = []

              if phases == "proj":
                  # ablation: projections only (startup + V + qkT fills)
                  for t in range(ST):
                      if t + 2 < ST:
                          fills.add(make_pair_fills(t + 2), t + 2)
                  fills.drain()
                  return

              for t in range(ST):
                  if t + 2 < ST:
                      fills.add(make_pair_fills(t + 2), t + 2)
                  fills.drain_through(t)
                  fills.budget = 24
                  if phases != "all":
                      pass
                  elif t in (4, 5, 6):
                      # prefetch W_out on SP while it's nearly idle
                      for k in range(3 * (t - 4), 3 * (t - 4) + (3 if t < 6 else 2)):
                          w = r8.tile([P, D], mdt, name=f"wout{k}", tag="r8")
                          nc.sync.dma_start(
                              out=w, in_=wo_d[k * P : (k + 1) * P, :]
                          )
                          wout.append(w)
                  # last pair's rs only feeds the on-chip PE broadcast, so a
                  # 16-bit dtype keeps that matmul at 1 cycle/row
                  rs = rsp.tile([P, S], bf16 if t == ST - 1 else f32,
                                name="rs", tag="rs")
                  for hh in range(2):
                      h = 2 * t + hh
                      po = 64 * hh
                      if hh == 0:
                          att[t] = attp.tile([P, S], mdt, name=f"att{t}", tag="att")
                      pso_c = [
                          pso.tile([P, HB], f32, name=f"pso{c}", tag="pso")
                          for c in range(2)
                      ]
                      # per-j pipeline: 1 wide scores matmul + mask + exp,
                      # then the (up to 2) PV half-matmuls; depth-2 lookahead
                      # (2 two-bank pss slots in flight)
                      depth = 2
                      pend = []
                      for j in range(ST):
                          et = emit_scores_j(t, po, j)
                          if hh == 1 and j in (2, 4) and last_norms:
                              last_norms.pop(0)()
                          pend.append((j, et))
                          if len(pend) > depth:
                              pj, pet = pend.pop(0)
                              fills.pop(1)
                              emit_pv(t, h, pj, pet, pso_c)
                              if pj == 3:
                                  emit_evict(t, po, 0, pso_c, rs)
                              fills.pop(1)
                      while pend:
                          pj, pet = pend.pop(0)
                          emit_pv(t, h, pj, pet, pso_c)
                          if pj == 3:
                              emit_evict(t, po, 0, pso_c, rs)
                      emit_evict(t, po, 1, pso_c, rs)
                      fills.pop(2)
                      if t == ST - 1:
                          # last pair: on-chip normalize (PE broadcast of the
                          # recip row + DVE multiply) — no DRAM round-trip on
                          # the critical tail. Deferred: head A's broadcasts
                          # run under head B's j-loop, head B's under
                          # out_proj's first k-loop, so the PE never waits on
                          # the DVE recip chain.
                          def mk_norm(po=po, rs=rs, c=0):
                              rb = po // 2
                              cs = c * HB
                              # psg is free in the tail (fills are done)
                              ps_bc = psg.tile(
                                  [P, HB], f32, name="ps_bc", tag="psg"
                              )
                              nc.tensor.matmul(
                                  ps_bc[0:64, :],
                                  ones_b[rb : rb + 1, 0:64],
                                  rs[rb : rb + 1, cs : cs + HB],
                                  start=True,
                                  stop=True,
                              )
                              nc.vector.tensor_mul(
                                  att[t][po : po + 64, cs : cs + HB],
                                  att[t][po : po + 64, cs : cs + HB],
                                  ps_bc[0:64, :],
                              )
                          last_norms.append(
                              lambda po=po, rs=rs, f=mk_norm: f(po, rs, 0)
                          )
                          last_norms.append(
                              lambda po=po, rs=rs, f=mk_norm: f(po, rs, 1)
                          )
                  if t < ST - 1:
                      emit_pair_norm(t, rs)
              fills.drain()
              if phases != "all":
                  return

              # ---- output projection: one 1024-wide accumulation per m-tile
              # (2-bank pss slots; m and m+1 pipeline across the 2 bufs)
              for m in range(ST):
                  ob = obp.tile([P, D], f32, name="ob", tag="ob")
                  ps_f = pss.tile([P, D], f32, name="ps_f", tag="pss")
                  for k in range(DT):
                      if k in (3, 6) and last_norms:
                          last_norms.pop(0)()
                      for c in range(2):
                          nc.tensor.matmul(
                              ps_f[:, c * HB : (c + 1) * HB],
                              att[k][:, m * P : (m + 1) * P],
                              wout[k][:, c * HB : (c + 1) * HB],
                              start=(k == 0),
                              stop=(k == DT - 1),
                          )
                  nc.vector.tensor_add(ob, ps_f, bias_o)
                  nc.sync.dma_start(
                      out=out_d[m * P : (m + 1) * P, :], in_=ob
                  )

            for _ in range(reps):
                one_pass()

    nc.compile()
    return nc


def get_nc(mm_dtype_name="bfloat16", reps=1, phases="all"):
    key = (mm_dtype_name, reps, phases)
    if key not in _NC_CACHE:
        _NC_CACHE[key] = _build_nc(mm_dtype_name, reps, phases)
    return _NC_CACHE[key]


DEFAULT_MM_DTYPE = "bfloat16"


def _np_mm_dtype(name):
    if name == "bfloat16":
        import ml_dtypes

        return ml_dtypes.bfloat16
    return np.float32


def kernel(hidden_states, W_attn, b_attn, W_out, b_out, _trace=False):
    from concourse.bass_utils import run_bass_kernel_spmd

    nc = get_nc(DEFAULT_MM_DTYPE)
    mdt = _np_mm_dtype(DEFAULT_MM_DTYPE)
    hidden_states = np.ascontiguousarray(hidden_states, dtype=mdt)
    W_attn = np.ascontiguousarray(W_attn, dtype=mdt)
    W_out = np.ascontiguousarray(W_out, dtype=mdt)
    in_maps = [
        {
            "hidden_states": hidden_states[b],
            "W_attn": W_attn,
            "b_attn": np.asarray(b_attn, np.float32),
            "W_out": W_out,
            "b_out": np.asarray(b_out, np.float32),
        }
        for b in range(N_CORES)
    ]
    res = run_bass_kernel_spmd(
        nc, in_maps, core_ids=list(range(N_CORES)), trace=_trace
    )
    out = np.stack([res.results[b]["out"] for b in range(N_CORES)], axis=0)
    if _trace:
        kernel.last_results = res
    return out

